# revision 33
# baseline (speedup 1.0000x reference)
"""Trainium2 Bass kernel for nn_ExtractionLayer (v8, stacked v-tiles).

metric[b,v,f] = sum_p amp[b,f,p] * exp(-c*(vol[v]*filt[f] - q[b,p])^2)
  amp = softmax_p(logits[b,f,p]),  c = 0.5/(sigma+0.001)^2

Sharding: data-parallel over batch B=32 -> 4 b's per core on 8 cores
(batch->core assignment optimized to balance the shared schedule).

Pieces = (f, 32-v tile of vol-sorted v); 4 pieces with adjacent
q-windows stack on the 128 PSUM partitions of one matmul chunk-group.
Per chunk-group centering lets 2-way bf16 splits reach ~17 bits:
S = X^2 - 2XQ + Q^2 - lnamp/c via a K=13 matmul.  ACT does
E=exp(-c*S) fp16, DVE folds col pairs (one 2x-mode tensor_tensor per
group), host sums the variable-count pair partials with
np.add.reduceat, so per-(b,chunk) column counts need no global max.

Columns pack densely into 512-col PSUM banks (chunk-groups split at
4-col boundaries across banks; bands = group-local bank index, PE
tile_position rows 0/32/64/96).  Input DRAM tensors are band-major;
DMA pieces are per (band, group-range) so each group's data lands
just-in-time across the sync/gpsimd/scalar queues (per-queue DMA
streams ~25-45 GB/s, so the stream must start small and stay ahead).
First group is 2 banks for a fast pipeline fill.
"""

import sys

for _p in ("/opt/trn_rl_repo", "/root/.axon_site/_ro/trn_rl_repo"):
    if _p not in sys.path:
        sys.path.append(_p)

import hashlib

import numpy as np
import ml_dtypes

BF16 = ml_dtypes.bfloat16

B, V, F, P = 32, 256, 128, 64
NCORES = 8
B_LOC = B // NCORES
T = 32                  # v rows per piece
S = 128 // T            # pieces per chunk-group
NK = 7 + S              # matmul contraction rows
BANK = 512              # psum cols per bank
GRAN = 4                # reduce segment width (folded to pairs on-chip)
THR_LN = 4.0            # window: c*(x-q)^2 <= THR_LN at the edge
PAD_PHI = 100.0         # pad cols -> exp(-c*100) == 0

_cache: dict = {}


def _split2(v):
    """2-way bf16 split: h + m ~= v to ~17 bits."""
    h = v.astype(BF16)
    m = (v - h.astype(np.float64)).astype(BF16)
    return h.astype(np.float32), m.astype(np.float32)


def _ceilg(n):
    return -(-n // GRAN) * GRAN


_ceil4 = _ceilg


class Schedule:
    """Core-independent processing plan (shared NEFF across 8 cores)."""

    def __init__(self, cgs, assign):
        self.cgs = cgs
        self.assign = assign
        ncg = len(cgs)
        w_b = np.stack([_ceil4(cg["nb"]) for cg in cgs])        # (ncg, 32)
        core_w = w_b[:, assign.reshape(-1)].reshape(ncg, 8, 4).sum(2)
        self.wcg = core_w.max(1)                                 # (ncg,)

        # ---- pack cg columns bin-major into banks of 512 ----
        pieces = []          # (cg, bin, off, lo, hi) covering cg-local [lo,hi)
        bin_i, off = 0, 0
        for ci in range(ncg):
            w = int(self.wcg[ci])
            lo = 0
            while w > 0:
                take = min(w, BANK - off)
                pieces.append((ci, bin_i, off, lo, lo + take))
                off += take
                lo += take
                w -= take
                if off == BANK:
                    bin_i, off = bin_i + 1, 0
        if off > 0:
            ci, bi, o, lo, hi = pieces[-1]
            pieces[-1] = (ci, bi, o, lo, hi + (BANK - off))   # pad-extend
            bin_i += 1
        self.nbins = bin_i
        self.rtot = self.nbins * BANK // 2    # pair partials

        # group sizes: ramp [1, 2, 4, 4, ...] for fast pipeline fill
        sizes = []
        left = self.nbins
        for want in (1, 2) if self.nbins > 8 else ():
            sizes.append(min(want, left))
            left -= sizes[-1]
        while left > 0:
            sizes.append(min(4, left))
            left -= sizes[-1]
        self.ngroups = len(sizes)
        self.nbg = sizes
        cum = np.concatenate([[0], np.cumsum(sizes)])
        bin_g = np.searchsorted(cum, np.arange(self.nbins), side="right") - 1
        bin_beta = np.arange(self.nbins) - cum[bin_g]

        # ---- band-major DRAM layouts ----
        # wmv: bin -> dram col block, ordered by (beta, g)
        order = sorted(range(self.nbins), key=lambda bi: (bin_beta[bi],
                                                          bin_g[bi]))
        self.wmv_dbase = np.empty(self.nbins, np.int64)
        for k, bi in enumerate(order):
            self.wmv_dbase[bi] = k * BANK
        self.wtot = self.nbins * BANK

        # stationary slots: sbuf per-band sequential; dram band-major
        self.band_nslot = [0, 0, 0, 0]
        piece_slot = []
        piece_gn = [{}, {}, {}, {}]      # beta -> {g: nslots}
        for ci, bi, o, lo, hi in pieces:
            beta = int(bin_beta[bi])
            g = int(bin_g[bi])
            piece_slot.append(self.band_nslot[beta])
            self.band_nslot[beta] += 1
            piece_gn[beta][g] = piece_gn[beta].get(g, 0) + 1
        xbase = np.concatenate([[0], np.cumsum(self.band_nslot)])
        self.xst_fill = []   # (dram_slot, ci)
        self.mm = [[] for _ in range(self.ngroups)]  # (beta, off, w, bslot)
        for k, (ci, bi, o, lo, hi) in enumerate(pieces):
            g, beta = int(bin_g[bi]), int(bin_beta[bi])
            bslot = piece_slot[k]
            dslot = int(xbase[beta]) + bslot
            self.xst_fill.append((dslot, ci))
            self.mm[g].append((beta, o, hi - lo, bslot))
        self.nslots = int(xbase[4])
        self.xtot = self.nslots * 128
        for g in range(self.ngroups):
            self.mm[g].sort(key=lambda t: (t[1], t[0]))

        # DMA pieces per band: group ranges [0],[1],[2,3],[4..]
        # xst entries (beta, dslot0, nslots, bslot0); wmv entries
        # (beta, dcol0, ncols, g0, ngrange)
        self.xst_dma = [[] for _ in range(4)]
        self.wmv_dma = [[] for _ in range(4)]
        for beta in range(4):
            gs = sorted(piece_gn[beta])          # groups present
            gcuts = [g for g in (gs[0], gs[0] + 1, gs[0] + 2, gs[0] + 4)
                     if g <= gs[-1]] + [gs[-1] + 1]
            gcuts = sorted(set(gcuts))
            # xst pieces
            spos = 0
            for a, b in zip(gcuts, gcuts[1:]):
                n = sum(piece_gn[beta].get(g, 0) for g in range(a, b))
                if n > 0:
                    self.xst_dma[beta].append(
                        (beta, int(xbase[beta]) + spos, n, spos))
                    spos += n
            # wmv pieces (band has one bank per present group, g-contig)
            dbase = None
            for bi in range(self.nbins):
                if int(bin_beta[bi]) == beta:
                    dbase = int(self.wmv_dbase[bi])
                    break
            for a, b in zip(gcuts, gcuts[1:]):
                ng = sum(1 for g in range(a, b) if g in piece_gn[beta])
                if ng > 0:
                    off_g = a - gs[0]
                    self.wmv_dma[beta].append(
                        (beta, dbase + off_g * BANK, ng * BANK, a, ng))

        # cg-local col -> (wmv dram col, psum col)
        self.gcol = [np.empty(int(self.wcg[ci]), np.int64)
                     for ci in range(ncg)]
        self.pcol = [np.empty(int(self.wcg[ci]), np.int64)
                     for ci in range(ncg)]
        for ci, bi, o, lo, hi in pieces:
            n = min(hi, int(self.wcg[ci])) - lo
            if n > 0:
                ar = np.arange(n)
                self.gcol[ci][lo:lo + n] = self.wmv_dbase[bi] + o + ar
                self.pcol[ci][lo:lo + n] = bi * BANK + o + ar

        # R cumulative base per group (pair cols)
        self.rbase = [int(cum[g]) * BANK // 2 for g in range(self.ngroups + 1)]

        # ---- per-core unpack runs (reduceat over pair cols) ----
        self.runs = []
        for core in range(8):
            bs = assign[core]
            ev = []
            for ci, cg in enumerate(cgs):
                pos = 0
                for lb in range(4):
                    n = int(cg["nb"][bs[lb]])
                    if n == 0:
                        continue
                    k = _ceil4(n) // GRAN
                    first = True
                    p0 = pos // GRAN
                    while k > 0:
                        gc = self.pcol[ci][p0 * GRAN]
                        run = 1
                        while (run < k and
                               self.pcol[ci][(p0 + run) * GRAN] ==
                               gc + run * GRAN):
                            run += 1
                        ev.append((int(gc) // 2, run * GRAN // 2,
                                   ci * 4 + lb, first))
                        first = False
                        p0 += run
                        k -= run
                    pos += _ceil4(n)
            ev.sort()
            starts, owners, firsts = [], [], []
            cur_end = 0
            for st, ln, ow, fi in ev:
                if st > cur_end:
                    starts.append(cur_end)
                    owners.append(-1)
                    firsts.append(True)
                starts.append(st)
                owners.append(ow)
                firsts.append(fi)
                cur_end = st + ln
            if cur_end < self.rtot:
                starts.append(cur_end)
                owners.append(-1)
                firsts.append(True)
            self.runs.append((np.array(starts), np.array(owners),
                              np.array(firsts)))

        h = hashlib.md5(repr((NK, THR_LN, self.nbins, tuple(sizes),
                              tuple(self.wcg.tolist()),
                              tuple(pieces))).encode()).hexdigest()
        self.key = h


def _build(minus_c, sched):
    import concourse.tile as tile
    from concourse import bacc, mybir

    fp32 = mybir.dt.float32
    fp16 = mybir.dt.float16
    bf16 = mybir.dt.bfloat16
    AF = mybir.ActivationFunctionType
    OP = mybir.AluOpType
    import concourse.bass as bass

    nc = bacc.Bacc("TRN2", target_bir_lowering=False, debug=False,
                   num_devices=NCORES)

    d_xst = nc.dram_tensor("xst", [NK, sched.xtot], bf16,
                           kind="ExternalInput")
    d_wmv = nc.dram_tensor("wmv", [NK, sched.wtot], bf16,
                           kind="ExternalInput")
    d_out = nc.dram_tensor("out", [128, sched.rtot], fp16,
                           kind="ExternalOutput")

    ngroups = sched.ngroups
    PMAX = 96 + NK

    with tile.TileContext(nc) as tc:
        with (
            tc.tile_pool(name="const", bufs=1) as cp,
            tc.tile_pool(name="ering", bufs=3) as ep,
            tc.tile_pool(name="psS", bufs=2, space=bass.MemorySpace.PSUM) as psS,
        ):
            warm = cp.tile([1, 2], fp32, tag="warm")
            nc.vector.memset(warm[:, :], 0.0)
            zb = cp.tile([128, 1], fp32, tag="zb")
            nc.vector.memset(zb[:, :], 0.0)
            nc.scalar.activation(warm[:, 0:1], warm[:, 1:2], AF.Exp,
                                 bias=zb[0:1, 0:1])

            xst = cp.tile([PMAX, max(sched.band_nslot) * 128], bf16,
                          tag="xst")
            wmv = cp.tile([PMAX, ngroups * BANK], bf16, tag="wmv")
            R = cp.tile([128, sched.rtot], fp16, tag="R")

            def x_issue(eng, x):
                beta, d0, n, b0 = x
                eng.dma_start(
                    xst[32 * beta:32 * beta + NK,
                        b0 * 128:(b0 + n) * 128],
                    d_xst.ap()[:, d0 * 128:(d0 + n) * 128])

            def w_issue(eng, wx):
                beta, d0, nc_, g0, ng = wx
                eng.dma_start(
                    wmv[32 * beta:32 * beta + NK,
                        g0 * BANK:(g0 + ng) * BANK],
                    d_wmv.ap()[:, d0:d0 + nc_])

            # piece-index-major issue: all [g0] pieces, then [g1], ...
            # early pieces spread over sync/gpsimd/scalar, rest over 2
            q3 = [nc.sync, nc.gpsimd, nc.scalar]
            q2 = [nc.sync, nc.gpsimd]
            k = 0
            for pi in range(4):
                for beta in range(4):
                    for lst in (sched.xst_dma, sched.wmv_dma):
                        if pi < len(lst[beta]):
                            if pi < 3:
                                eng = q3[k % 3]
                            else:
                                eng = q2[k % 2]
                            k += 1
                            if lst is sched.xst_dma:
                                x_issue(eng, lst[beta][pi])
                            else:
                                w_issue(eng, lst[beta][pi])

            ocur = 0
            oi = 0
            for g in range(ngroups):
                nbg = sched.nbg[g]
                h = nbg * BANK
                sS = psS.tile([128, 4 * BANK], fp32, tag="S", name="sS")
                for beta, off, w, bslot in sched.mm[g]:
                    r0 = 32 * beta
                    nc.tensor.matmul(
                        sS[:, beta * BANK + off:beta * BANK + off + w],
                        xst[r0:r0 + NK, bslot * 128:(bslot + 1) * 128],
                        wmv[r0:r0 + NK, g * BANK + off:g * BANK + off + w],
                        start=True, stop=True,
                        tile_position=(r0, 0),
                    )
                E = ep.tile([128, 4 * BANK], fp16, tag="E", name="E")
                nc.scalar.activation(E[:, 0:h], sS[:, 0:h], AF.Exp,
                                     scale=float(minus_c), bias=zb[:, 0:1])
                Ev = E[:, 0:h].rearrange("p (s x) -> p s x", x=GRAN)
                rb = sched.rbase[g]
                Rp = (R[:, rb:rb + h // 2]
                      .rearrange("p (s x) -> p s x", x=GRAN // 2))
                half = GRAN // 2
                nc.vector.tensor_tensor(Rp, Ev[:, :, 0:half],
                                        Ev[:, :, half:GRAN], OP.add)
                rend = sched.rbase[g + 1]
                if g % 2 == 1 or g == ngroups - 1:
                    q2[oi % 2].dma_start(d_out.ap()[:, ocur:rend],
                                         R[:, ocur:rend])
                    oi += 1
                    ocur = rend

    nc.compile()
    return nc


def _get_nc(minus_c, sched):
    key = (float(minus_c), sched.key)
    if key not in _cache:
        _cache[key] = _build(minus_c, sched)
    return _cache[key]


def _make_schedule(c, q, xs):
    """Pieces -> chunk-groups -> batch assignment -> Schedule."""
    thr = np.sqrt(THR_LN / c)
    pieces = []
    for f in range(F):
        for t in range(V // T):
            xw = xs[t * T:(t + 1) * T, f]
            pieces.append((float(xw.min() + xw.max()) / 2, f, t))
    pieces.sort()
    cgs = []
    for i in range(0, len(pieces), S):
        grp = pieces[i:i + S]
        los, his = [], []
        for _, f, t in grp:
            xw = xs[t * T:(t + 1) * T, f]
            los.append(xw.min())
            his.append(xw.max())
        lo, hi = min(los) - thr, max(his) + thr
        sel = (q >= lo) & (q <= hi)
        nb = sel.sum(axis=1).astype(np.int64)
        if nb.max() == 0:
            continue
        cgs.append({"m": (lo + hi) / 2, "fs": [f for _, f, _ in grp],
                    "ts": [t for _, _, t in grp], "sel": sel, "nb": nb})

    # batch -> core assignment: minimize sum_cg max_core width
    w_b = np.stack([_ceil4(cg["nb"]) for cg in cgs])            # (ncg, 32)
    tot = w_b.sum(0)
    order = np.argsort(-tot, kind="stable")
    assign = [[] for _ in range(8)]
    loads = np.zeros(8)
    for b in order:
        ci = int(np.argmin([loads[i] if len(assign[i]) < 4 else 1e18
                            for i in range(8)]))
        assign[ci].append(int(b))
        loads[ci] += tot[b]
    assign = np.array(assign)

    def obj(a):
        return w_b[:, a.reshape(-1)].reshape(-1, 8, 4).sum(2).max(1).sum()

    rng = np.random.RandomState(0)
    best = obj(assign)
    for _ in range(20000):
        c1, c2 = rng.randint(8), rng.randint(8)
        if c1 == c2:
            continue
        i1, i2 = rng.randint(4), rng.randint(4)
        a2 = assign.copy()
        a2[c1, i1], a2[c2, i2] = assign[c2, i2], assign[c1, i1]
        o2 = obj(a2)
        if o2 < best:
            best, assign = o2, a2
    return Schedule(cgs, assign)


def kernel(q2_obs_scaled, amplitude_logits, volumes, filters, sigma,
           _trace=False, _tmpdir=None):
    from concourse.bass_utils import run_bass_kernel_spmd

    sig = float(np.asarray(sigma).reshape(()))
    minus_c = -0.5 / (sig + 0.001) ** 2
    c = -minus_c

    q = np.asarray(q2_obs_scaled, np.float64)                    # (B, P)
    lg = np.asarray(amplitude_logits, np.float64).reshape(B, F, P)
    vol = np.asarray(volumes, np.float64).reshape(V)
    fil = np.asarray(filters, np.float64).reshape(F)

    mx = lg.max(axis=2, keepdims=True)
    lnamp = lg - (mx + np.log(np.exp(lg - mx).sum(axis=2, keepdims=True)))

    vperm = np.argsort(vol, kind="stable")
    vs = vol[vperm]
    xs = vs[:, None] * fil[None, :]                              # (V, F)

    sched = _make_schedule(c, q, xs)
    nc = _get_nc(minus_c, sched)
    cgs = sched.cgs
    ncg = len(cgs)

    # ---- stationary tile (shared by all cores) ----
    xst = np.zeros((NK, sched.xtot), dtype=BF16)
    ones_j = np.zeros((S, 128), dtype=BF16)
    for j in range(S):
        ones_j[j, j * T:(j + 1) * T] = 1.0
    xrows_cg = {}
    for ci, cg in enumerate(cgs):
        X = np.concatenate([xs[t * T:(t + 1) * T, f] - cg["m"]
                            for f, t in zip(cg["fs"], cg["ts"])])
        X2h, X2m = _split2(X * X)
        Xh, Xm = _split2(X)
        xrows_cg[ci] = (X2h, X2m, Xh, Xm)
    for dslot, ci in sched.xst_fill:
        X2h, X2m, Xh, Xm = xrows_cg[ci]
        c0 = dslot * 128
        xst[0, c0:c0 + 128] = X2h
        xst[1, c0:c0 + 128] = X2m
        xst[2, c0:c0 + 128] = Xh
        xst[3, c0:c0 + 128] = Xh
        xst[4, c0:c0 + 128] = Xm
        xst[5, c0:c0 + 128] = 1.0
        xst[6, c0:c0 + 128] = 1.0
        for j in range(S):
            xst[7 + j, c0:c0 + 128] = ones_j[j]

    # ---- per-cg moving data for all 32 batches ----
    cg_data = []
    for ci, cg in enumerate(cgs):
        bi_, pi_ = np.nonzero(cg["sel"])          # b-major, p ascending
        Q = q[bi_, pi_] - cg["m"]
        Wh, Wm = _split2(-2.0 * Q)
        Q2h, Q2m = _split2(Q * Q)
        Ls = [(-lnamp[bi_, f, pi_] / c).astype(BF16).astype(np.float32)
              for f in cg["fs"]]
        off_b = np.zeros(B + 1, np.int64)
        np.cumsum(np.bincount(bi_, minlength=B), out=off_b[1:])
        cg_data.append((Wh, Wm, Q2h, Q2m, Ls, off_b))

    # ---- per-core moving tiles ----
    in_maps = []
    for core in range(NCORES):
        wmv = np.zeros((NK, sched.wtot), dtype=BF16)
        wmv[0:2] = 1.0
        wmv[5:NK] = PAD_PHI
        bs = sched.assign[core]
        for ci in range(ncg):
            Wh, Wm, Q2h, Q2m, Ls, off_b = cg_data[ci]
            gcol = sched.gcol[ci]
            pos = 0
            for lb in range(4):
                b = int(bs[lb])
                n = int(cgs[ci]["nb"][b])
                if n == 0:
                    continue
                seg = slice(off_b[b], off_b[b] + n)
                gc = gcol[pos:pos + n]
                wmv[2, gc] = Wh[seg]
                wmv[3, gc] = Wm[seg]
                wmv[4, gc] = Wh[seg]
                wmv[5, gc] = Q2h[seg]
                wmv[6, gc] = Q2m[seg]
                for j in range(S):
                    wmv[7 + j, gc] = Ls[j][seg]
                pos += _ceil4(n)
        in_maps.append({"xst": xst, "wmv": wmv})

    kw = {}
    if _trace:
        kw = {"trace": True, "tmpdir": _tmpdir}
    res = run_bass_kernel_spmd(nc, in_maps, core_ids=list(range(NCORES)), **kw)

    # ---- host unpack ----
    dest = np.empty((ncg, 128), np.int64)
    for ci, cg in enumerate(cgs):
        for j, (f, t) in enumerate(zip(cg["fs"], cg["ts"])):
            dest[ci, j * T:(j + 1) * T] = vperm[t * T:(t + 1) * T] * F + f
    out = np.zeros((B, V * F), dtype=np.float64)
    for core in range(NCORES):
        Rr = np.asarray(res.results[core]["out"], np.float16)
        P32 = Rr.astype(np.float32)
        starts, owners, firsts = sched.runs[core]
        red = np.add.reduceat(P32, starts, axis=1)
        vals = np.zeros((128, ncg * 4), np.float32)
        dm = (owners >= 0) & firsts
        vals[:, owners[dm]] = red[:, dm]
        for r in np.nonzero((owners >= 0) & ~firsts)[0]:
            vals[:, owners[r]] += red[:, r]
        v3 = vals.reshape(128, ncg, 4)
        bs = sched.assign[core]
        for lb in range(4):
            out[bs[lb], dest.reshape(-1)] = v3[:, :, lb].T.reshape(-1)
    out = out.reshape(B, V, F).astype(np.float32)
    if _trace:
        return out, res
    return out


# revision 34
# speedup vs baseline: 1.0432x; 1.0432x over previous
"""Trainium2 Bass kernel for nn_ExtractionLayer (v8, stacked v-tiles).

metric[b,v,f] = sum_p amp[b,f,p] * exp(-c*(vol[v]*filt[f] - q[b,p])^2)
  amp = softmax_p(logits[b,f,p]),  c = 0.5/(sigma+0.001)^2

Sharding: data-parallel over batch B=32 -> 4 b's per core on 8 cores
(batch->core assignment optimized to balance the shared schedule).

Pieces = (f, 32-v tile of vol-sorted v); 4 pieces with adjacent
q-windows stack on the 128 PSUM partitions of one matmul chunk-group.
Per chunk-group centering lets 2-way bf16 splits reach ~17 bits:
S = X^2 - 2XQ + Q^2 - lnamp/c via a K=13 matmul.  ACT does
E=exp(-c*S) fp16, DVE folds col pairs (one 2x-mode tensor_tensor per
group), host sums the variable-count pair partials with
np.add.reduceat, so per-(b,chunk) column counts need no global max.

Columns pack densely into 512-col PSUM banks (chunk-groups split at
4-col boundaries across banks; bands = group-local bank index, PE
tile_position rows 0/32/64/96).  Input DRAM tensors are band-major;
DMA pieces are per (band, group-range) so each group's data lands
just-in-time across the sync/gpsimd/scalar queues (per-queue DMA
streams ~25-45 GB/s, so the stream must start small and stay ahead).
First group is 2 banks for a fast pipeline fill.
"""

import sys

for _p in ("/opt/trn_rl_repo", "/root/.axon_site/_ro/trn_rl_repo"):
    if _p not in sys.path:
        sys.path.append(_p)

import hashlib

import numpy as np
import ml_dtypes

BF16 = ml_dtypes.bfloat16

B, V, F, P = 32, 256, 128, 64
NCORES = 8
B_LOC = B // NCORES
T = 32                  # v rows per piece
S = 128 // T            # pieces per chunk-group
NK = 7 + S              # matmul contraction rows
BANK = 512              # psum cols per bank
GRAN = 4                # reduce segment width (folded to pairs on-chip)
THR_LN = 4.0            # window: c*(x-q)^2 <= THR_LN at the edge
PAD_PHI = 100.0         # pad cols -> exp(-c*100) == 0

_cache: dict = {}


def _split2(v):
    """2-way bf16 split: h + m ~= v to ~17 bits."""
    h = v.astype(BF16)
    m = (v - h.astype(np.float64)).astype(BF16)
    return h.astype(np.float32), m.astype(np.float32)


def _ceilg(n):
    return -(-n // GRAN) * GRAN


_ceil4 = _ceilg


class Schedule:
    """Core-independent processing plan (shared NEFF across 8 cores)."""

    def __init__(self, cgs, assign):
        self.cgs = cgs
        self.assign = assign
        ncg = len(cgs)
        w_b = np.stack([_ceil4(cg["nb"]) for cg in cgs])        # (ncg, 32)
        core_w = w_b[:, assign.reshape(-1)].reshape(ncg, 8, 4).sum(2)
        self.wcg = core_w.max(1)                                 # (ncg,)

        # ---- pack cg columns bin-major into banks of 512 ----
        pieces = []          # (cg, bin, off, lo, hi) covering cg-local [lo,hi)
        bin_i, off = 0, 0
        for ci in range(ncg):
            w = int(self.wcg[ci])
            lo = 0
            while w > 0:
                take = min(w, BANK - off)
                pieces.append((ci, bin_i, off, lo, lo + take))
                off += take
                lo += take
                w -= take
                if off == BANK:
                    bin_i, off = bin_i + 1, 0
        if off > 0:
            ci, bi, o, lo, hi = pieces[-1]
            pieces[-1] = (ci, bi, o, lo, hi + (BANK - off))   # pad-extend
            bin_i += 1
        self.nbins = bin_i
        self.rtot = self.nbins * BANK // 2    # pair partials

        # group sizes: ramp [1, 2, 4, 4, ...] for fast pipeline fill
        sizes = []
        left = self.nbins
        for want in (1, 2) if self.nbins > 8 else ():
            sizes.append(min(want, left))
            left -= sizes[-1]
        while left > 0:
            sizes.append(min(4, left))
            left -= sizes[-1]
        self.ngroups = len(sizes)
        self.nbg = sizes
        cum = np.concatenate([[0], np.cumsum(sizes)])
        bin_g = np.searchsorted(cum, np.arange(self.nbins), side="right") - 1
        bin_beta = np.arange(self.nbins) - cum[bin_g]

        # ---- band-major DRAM layouts ----
        # wmv: bin -> dram col block, ordered by (beta, g)
        order = sorted(range(self.nbins), key=lambda bi: (bin_beta[bi],
                                                          bin_g[bi]))
        self.wmv_dbase = np.empty(self.nbins, np.int64)
        for k, bi in enumerate(order):
            self.wmv_dbase[bi] = k * BANK
        self.wtot = self.nbins * BANK

        # stationary slots: sbuf per-band sequential; dram band-major
        self.band_nslot = [0, 0, 0, 0]
        piece_slot = []
        piece_gn = [{}, {}, {}, {}]      # beta -> {g: nslots}
        for ci, bi, o, lo, hi in pieces:
            beta = int(bin_beta[bi])
            g = int(bin_g[bi])
            piece_slot.append(self.band_nslot[beta])
            self.band_nslot[beta] += 1
            piece_gn[beta][g] = piece_gn[beta].get(g, 0) + 1
        xbase = np.concatenate([[0], np.cumsum(self.band_nslot)])
        self.xst_fill = []   # (dram_slot, ci)
        self.mm = [[] for _ in range(self.ngroups)]  # (beta, off, w, bslot)
        for k, (ci, bi, o, lo, hi) in enumerate(pieces):
            g, beta = int(bin_g[bi]), int(bin_beta[bi])
            bslot = piece_slot[k]
            dslot = int(xbase[beta]) + bslot
            self.xst_fill.append((dslot, ci))
            self.mm[g].append((beta, o, hi - lo, bslot))
        self.nslots = int(xbase[4])
        self.xtot = self.nslots * 128
        for g in range(self.ngroups):
            self.mm[g].sort(key=lambda t: (t[1], t[0]))

        # DMA pieces per band: group ranges [0],[1],[2,3],[4..]
        # xst entries (beta, dslot0, nslots, bslot0); wmv entries
        # (beta, dcol0, ncols, g0, ngrange)
        self.xst_dma = [[] for _ in range(4)]
        self.wmv_dma = [[] for _ in range(4)]
        for beta in range(4):
            gs = sorted(piece_gn[beta])          # groups present
            gcuts = [g for g in (gs[0], gs[0] + 1, gs[0] + 2, gs[0] + 4)
                     if g <= gs[-1]] + [gs[-1] + 1]
            gcuts = sorted(set(gcuts))
            # xst pieces
            spos = 0
            for a, b in zip(gcuts, gcuts[1:]):
                n = sum(piece_gn[beta].get(g, 0) for g in range(a, b))
                if n > 0:
                    self.xst_dma[beta].append(
                        (beta, int(xbase[beta]) + spos, n, spos))
                    spos += n
            # wmv pieces (band has one bank per present group, g-contig)
            dbase = None
            for bi in range(self.nbins):
                if int(bin_beta[bi]) == beta:
                    dbase = int(self.wmv_dbase[bi])
                    break
            for a, b in zip(gcuts, gcuts[1:]):
                ng = sum(1 for g in range(a, b) if g in piece_gn[beta])
                if ng > 0:
                    off_g = a - gs[0]
                    self.wmv_dma[beta].append(
                        (beta, dbase + off_g * BANK, ng * BANK, a, ng))

        # cg-local col -> (wmv dram col, psum col)
        self.gcol = [np.empty(int(self.wcg[ci]), np.int64)
                     for ci in range(ncg)]
        self.pcol = [np.empty(int(self.wcg[ci]), np.int64)
                     for ci in range(ncg)]
        for ci, bi, o, lo, hi in pieces:
            n = min(hi, int(self.wcg[ci])) - lo
            if n > 0:
                ar = np.arange(n)
                self.gcol[ci][lo:lo + n] = self.wmv_dbase[bi] + o + ar
                self.pcol[ci][lo:lo + n] = bi * BANK + o + ar

        # R cumulative base per group (pair cols)
        self.rbase = [int(cum[g]) * BANK // 2 for g in range(self.ngroups + 1)]

        # ---- per-core unpack runs (reduceat over pair cols) ----
        self.runs = []
        for core in range(8):
            bs = assign[core]
            ev = []
            for ci, cg in enumerate(cgs):
                pos = 0
                for lb in range(4):
                    n = int(cg["nb"][bs[lb]])
                    if n == 0:
                        continue
                    k = _ceil4(n) // GRAN
                    first = True
                    p0 = pos // GRAN
                    while k > 0:
                        gc = self.pcol[ci][p0 * GRAN]
                        run = 1
                        while (run < k and
                               self.pcol[ci][(p0 + run) * GRAN] ==
                               gc + run * GRAN):
                            run += 1
                        ev.append((int(gc) // 2, run * GRAN // 2,
                                   ci * 4 + lb, first))
                        first = False
                        p0 += run
                        k -= run
                    pos += _ceil4(n)
            ev.sort()
            starts, owners, firsts = [], [], []
            cur_end = 0
            for st, ln, ow, fi in ev:
                if st > cur_end:
                    starts.append(cur_end)
                    owners.append(-1)
                    firsts.append(True)
                starts.append(st)
                owners.append(ow)
                firsts.append(fi)
                cur_end = st + ln
            if cur_end < self.rtot:
                starts.append(cur_end)
                owners.append(-1)
                firsts.append(True)
            self.runs.append((np.array(starts), np.array(owners),
                              np.array(firsts)))

        h = hashlib.md5(repr((NK, THR_LN, self.nbins, tuple(sizes),
                              tuple(self.wcg.tolist()),
                              tuple(pieces))).encode()).hexdigest()
        self.key = h


def _build(minus_c, sched):
    import concourse.tile as tile
    from concourse import bacc, mybir

    fp32 = mybir.dt.float32
    fp16 = mybir.dt.float16
    bf16 = mybir.dt.bfloat16
    AF = mybir.ActivationFunctionType
    OP = mybir.AluOpType
    import concourse.bass as bass

    nc = bacc.Bacc("TRN2", target_bir_lowering=False, debug=False,
                   num_devices=NCORES)

    d_xst = nc.dram_tensor("xst", [NK, sched.xtot], bf16,
                           kind="ExternalInput")
    d_wmv = nc.dram_tensor("wmv", [NK, sched.wtot], bf16,
                           kind="ExternalInput")
    d_out = nc.dram_tensor("out", [128, sched.rtot], fp16,
                           kind="ExternalOutput")

    ngroups = sched.ngroups
    PMAX = 96 + NK

    with tile.TileContext(nc) as tc:
        with (
            tc.tile_pool(name="const", bufs=1) as cp,
            tc.tile_pool(name="ering", bufs=3) as ep,
            tc.tile_pool(name="psS", bufs=2, space=bass.MemorySpace.PSUM) as psS,
        ):
            warm = cp.tile([1, 2], fp32, tag="warm")
            nc.vector.memset(warm[:, :], 0.0)
            zb = cp.tile([128, 1], fp32, tag="zb")
            nc.vector.memset(zb[:, :], 0.0)
            nc.scalar.activation(warm[:, 0:1], warm[:, 1:2], AF.Exp,
                                 bias=zb[0:1, 0:1])

            xst = cp.tile([PMAX, max(sched.band_nslot) * 128], bf16,
                          tag="xst")
            wmv = cp.tile([PMAX, ngroups * BANK], bf16, tag="wmv")
            R = cp.tile([128, sched.rtot], fp16, tag="R")

            def x_issue(eng, x):
                beta, d0, n, b0 = x
                eng.dma_start(
                    xst[32 * beta:32 * beta + NK,
                        b0 * 128:(b0 + n) * 128],
                    d_xst.ap()[:, d0 * 128:(d0 + n) * 128])

            def w_issue(eng, wx):
                beta, d0, nc_, g0, ng = wx
                eng.dma_start(
                    wmv[32 * beta:32 * beta + NK,
                        g0 * BANK:(g0 + ng) * BANK],
                    d_wmv.ap()[:, d0:d0 + nc_])

            # piece-index-major issue: all [g0] pieces, then [g1], ...
            # early pieces spread over sync/gpsimd/scalar, rest over 2
            q3 = [nc.sync, nc.gpsimd, nc.scalar]
            q2 = [nc.sync, nc.gpsimd]
            k = 0
            for pi in range(4):
                for beta in range(4):
                    for lst in (sched.xst_dma, sched.wmv_dma):
                        if pi < len(lst[beta]):
                            if pi < 2:
                                eng = q3[k % 3]
                            else:
                                eng = q2[k % 2]
                            k += 1
                            if lst is sched.xst_dma:
                                x_issue(eng, lst[beta][pi])
                            else:
                                w_issue(eng, lst[beta][pi])

            ocur = 0
            oi = 0
            for g in range(ngroups):
                nbg = sched.nbg[g]
                h = nbg * BANK
                sS = psS.tile([128, 4 * BANK], fp32, tag="S", name="sS")
                for beta, off, w, bslot in sched.mm[g]:
                    r0 = 32 * beta
                    nc.tensor.matmul(
                        sS[:, beta * BANK + off:beta * BANK + off + w],
                        xst[r0:r0 + NK, bslot * 128:(bslot + 1) * 128],
                        wmv[r0:r0 + NK, g * BANK + off:g * BANK + off + w],
                        start=True, stop=True,
                        tile_position=(r0, 0),
                    )
                E = ep.tile([128, 4 * BANK], fp16, tag="E", name="E")
                nc.scalar.activation(E[:, 0:h], sS[:, 0:h], AF.Exp,
                                     scale=float(minus_c), bias=zb[:, 0:1])
                Ev = E[:, 0:h].rearrange("p (s x) -> p s x", x=GRAN)
                rb = sched.rbase[g]
                Rp = (R[:, rb:rb + h // 2]
                      .rearrange("p (s x) -> p s x", x=GRAN // 2))
                half = GRAN // 2
                nc.vector.tensor_tensor(Rp, Ev[:, :, 0:half],
                                        Ev[:, :, half:GRAN], OP.add)
                rend = sched.rbase[g + 1]
                if g % 2 == 1 or g == ngroups - 1:
                    q2[oi % 2].dma_start(d_out.ap()[:, ocur:rend],
                                         R[:, ocur:rend])
                    oi += 1
                    ocur = rend

    nc.compile()
    return nc


def _get_nc(minus_c, sched):
    key = (float(minus_c), sched.key)
    if key not in _cache:
        _cache[key] = _build(minus_c, sched)
    return _cache[key]


def _make_schedule(c, q, xs):
    """Pieces -> chunk-groups -> batch assignment -> Schedule."""
    thr = np.sqrt(THR_LN / c)
    pieces = []
    for f in range(F):
        for t in range(V // T):
            xw = xs[t * T:(t + 1) * T, f]
            pieces.append((float(xw.min() + xw.max()) / 2, f, t))
    pieces.sort()
    cgs = []
    for i in range(0, len(pieces), S):
        grp = pieces[i:i + S]
        los, his = [], []
        for _, f, t in grp:
            xw = xs[t * T:(t + 1) * T, f]
            los.append(xw.min())
            his.append(xw.max())
        lo, hi = min(los) - thr, max(his) + thr
        sel = (q >= lo) & (q <= hi)
        nb = sel.sum(axis=1).astype(np.int64)
        if nb.max() == 0:
            continue
        cgs.append({"m": (lo + hi) / 2, "fs": [f for _, f, _ in grp],
                    "ts": [t for _, _, t in grp], "sel": sel, "nb": nb})

    # batch -> core assignment: minimize sum_cg max_core width
    w_b = np.stack([_ceil4(cg["nb"]) for cg in cgs])            # (ncg, 32)
    tot = w_b.sum(0)
    order = np.argsort(-tot, kind="stable")
    assign = [[] for _ in range(8)]
    loads = np.zeros(8)
    for b in order:
        ci = int(np.argmin([loads[i] if len(assign[i]) < 4 else 1e18
                            for i in range(8)]))
        assign[ci].append(int(b))
        loads[ci] += tot[b]
    assign = np.array(assign)

    def obj(a):
        return w_b[:, a.reshape(-1)].reshape(-1, 8, 4).sum(2).max(1).sum()

    rng = np.random.RandomState(0)
    best = obj(assign)
    for _ in range(20000):
        c1, c2 = rng.randint(8), rng.randint(8)
        if c1 == c2:
            continue
        i1, i2 = rng.randint(4), rng.randint(4)
        a2 = assign.copy()
        a2[c1, i1], a2[c2, i2] = assign[c2, i2], assign[c1, i1]
        o2 = obj(a2)
        if o2 < best:
            best, assign = o2, a2
    return Schedule(cgs, assign)


def kernel(q2_obs_scaled, amplitude_logits, volumes, filters, sigma,
           _trace=False, _tmpdir=None):
    from concourse.bass_utils import run_bass_kernel_spmd

    sig = float(np.asarray(sigma).reshape(()))
    minus_c = -0.5 / (sig + 0.001) ** 2
    c = -minus_c

    q = np.asarray(q2_obs_scaled, np.float64)                    # (B, P)
    lg = np.asarray(amplitude_logits, np.float64).reshape(B, F, P)
    vol = np.asarray(volumes, np.float64).reshape(V)
    fil = np.asarray(filters, np.float64).reshape(F)

    mx = lg.max(axis=2, keepdims=True)
    lnamp = lg - (mx + np.log(np.exp(lg - mx).sum(axis=2, keepdims=True)))

    vperm = np.argsort(vol, kind="stable")
    vs = vol[vperm]
    xs = vs[:, None] * fil[None, :]                              # (V, F)

    sched = _make_schedule(c, q, xs)
    nc = _get_nc(minus_c, sched)
    cgs = sched.cgs
    ncg = len(cgs)

    # ---- stationary tile (shared by all cores) ----
    xst = np.zeros((NK, sched.xtot), dtype=BF16)
    ones_j = np.zeros((S, 128), dtype=BF16)
    for j in range(S):
        ones_j[j, j * T:(j + 1) * T] = 1.0
    xrows_cg = {}
    for ci, cg in enumerate(cgs):
        X = np.concatenate([xs[t * T:(t + 1) * T, f] - cg["m"]
                            for f, t in zip(cg["fs"], cg["ts"])])
        X2h, X2m = _split2(X * X)
        Xh, Xm = _split2(X)
        xrows_cg[ci] = (X2h, X2m, Xh, Xm)
    for dslot, ci in sched.xst_fill:
        X2h, X2m, Xh, Xm = xrows_cg[ci]
        c0 = dslot * 128
        xst[0, c0:c0 + 128] = X2h
        xst[1, c0:c0 + 128] = X2m
        xst[2, c0:c0 + 128] = Xh
        xst[3, c0:c0 + 128] = Xh
        xst[4, c0:c0 + 128] = Xm
        xst[5, c0:c0 + 128] = 1.0
        xst[6, c0:c0 + 128] = 1.0
        for j in range(S):
            xst[7 + j, c0:c0 + 128] = ones_j[j]

    # ---- per-cg moving data for all 32 batches ----
    cg_data = []
    for ci, cg in enumerate(cgs):
        bi_, pi_ = np.nonzero(cg["sel"])          # b-major, p ascending
        Q = q[bi_, pi_] - cg["m"]
        Wh, Wm = _split2(-2.0 * Q)
        Q2h, Q2m = _split2(Q * Q)
        Ls = [(-lnamp[bi_, f, pi_] / c).astype(BF16).astype(np.float32)
              for f in cg["fs"]]
        off_b = np.zeros(B + 1, np.int64)
        np.cumsum(np.bincount(bi_, minlength=B), out=off_b[1:])
        cg_data.append((Wh, Wm, Q2h, Q2m, Ls, off_b))

    # ---- per-core moving tiles ----
    in_maps = []
    for core in range(NCORES):
        wmv = np.zeros((NK, sched.wtot), dtype=BF16)
        wmv[0:2] = 1.0
        wmv[5:NK] = PAD_PHI
        bs = sched.assign[core]
        for ci in range(ncg):
            Wh, Wm, Q2h, Q2m, Ls, off_b = cg_data[ci]
            gcol = sched.gcol[ci]
            pos = 0
            for lb in range(4):
                b = int(bs[lb])
                n = int(cgs[ci]["nb"][b])
                if n == 0:
                    continue
                seg = slice(off_b[b], off_b[b] + n)
                gc = gcol[pos:pos + n]
                wmv[2, gc] = Wh[seg]
                wmv[3, gc] = Wm[seg]
                wmv[4, gc] = Wh[seg]
                wmv[5, gc] = Q2h[seg]
                wmv[6, gc] = Q2m[seg]
                for j in range(S):
                    wmv[7 + j, gc] = Ls[j][seg]
                pos += _ceil4(n)
        in_maps.append({"xst": xst, "wmv": wmv})

    kw = {}
    if _trace:
        kw = {"trace": True, "tmpdir": _tmpdir}
    res = run_bass_kernel_spmd(nc, in_maps, core_ids=list(range(NCORES)), **kw)

    # ---- host unpack ----
    dest = np.empty((ncg, 128), np.int64)
    for ci, cg in enumerate(cgs):
        for j, (f, t) in enumerate(zip(cg["fs"], cg["ts"])):
            dest[ci, j * T:(j + 1) * T] = vperm[t * T:(t + 1) * T] * F + f
    out = np.zeros((B, V * F), dtype=np.float64)
    for core in range(NCORES):
        Rr = np.asarray(res.results[core]["out"], np.float16)
        P32 = Rr.astype(np.float32)
        starts, owners, firsts = sched.runs[core]
        red = np.add.reduceat(P32, starts, axis=1)
        vals = np.zeros((128, ncg * 4), np.float32)
        dm = (owners >= 0) & firsts
        vals[:, owners[dm]] = red[:, dm]
        for r in np.nonzero((owners >= 0) & ~firsts)[0]:
            vals[:, owners[r]] += red[:, r]
        v3 = vals.reshape(128, ncg, 4)
        bs = sched.assign[core]
        for lb in range(4):
            out[bs[lb], dest.reshape(-1)] = v3[:, :, lb].T.reshape(-1)
    out = out.reshape(B, V, F).astype(np.float32)
    if _trace:
        return out, res
    return out


# revision 35
# speedup vs baseline: 1.0920x; 1.0468x over previous
"""Trainium2 Bass kernel for nn_ExtractionLayer (v8, stacked v-tiles).

metric[b,v,f] = sum_p amp[b,f,p] * exp(-c*(vol[v]*filt[f] - q[b,p])^2)
  amp = softmax_p(logits[b,f,p]),  c = 0.5/(sigma+0.001)^2

Sharding: data-parallel over batch B=32 -> 4 b's per core on 8 cores
(batch->core assignment optimized to balance the shared schedule).

Pieces = (f, 32-v tile of vol-sorted v); 4 pieces with adjacent
q-windows stack on the 128 PSUM partitions of one matmul chunk-group.
Per chunk-group centering lets 2-way bf16 splits reach ~17 bits:
S = X^2 - 2XQ + Q^2 - lnamp/c via a K=13 matmul.  ACT does
E=exp(-c*S) fp16, DVE folds col pairs (one 2x-mode tensor_tensor per
group), host sums the variable-count pair partials with
np.add.reduceat, so per-(b,chunk) column counts need no global max.

Columns pack densely into 512-col PSUM banks (chunk-groups split at
4-col boundaries across banks; bands = group-local bank index, PE
tile_position rows 0/32/64/96).  Input DRAM tensors are band-major;
DMA pieces are per (band, group-range) so each group's data lands
just-in-time across the sync/gpsimd/scalar queues (per-queue DMA
streams ~25-45 GB/s, so the stream must start small and stay ahead).
First group is 2 banks for a fast pipeline fill.
"""

import sys

for _p in ("/opt/trn_rl_repo", "/root/.axon_site/_ro/trn_rl_repo"):
    if _p not in sys.path:
        sys.path.append(_p)

import hashlib

import numpy as np
import ml_dtypes

BF16 = ml_dtypes.bfloat16

B, V, F, P = 32, 256, 128, 64
NCORES = 8
B_LOC = B // NCORES
T = 32                  # v rows per piece
S = 128 // T            # pieces per chunk-group
NK = 7 + S              # matmul contraction rows
BANK = 512              # psum cols per bank
GRAN = 4                # reduce segment width (folded to pairs on-chip)
THR_LN = 4.0            # window: c*(x-q)^2 <= THR_LN at the edge
PAD_PHI = 100.0         # pad cols -> exp(-c*100) == 0

_cache: dict = {}


def _split2(v):
    """2-way bf16 split: h + m ~= v to ~17 bits."""
    h = v.astype(BF16)
    m = (v - h.astype(np.float64)).astype(BF16)
    return h.astype(np.float32), m.astype(np.float32)


def _ceilg(n):
    return -(-n // GRAN) * GRAN


_ceil4 = _ceilg


class Schedule:
    """Core-independent processing plan (shared NEFF across 8 cores)."""

    def __init__(self, cgs, assign):
        self.cgs = cgs
        self.assign = assign
        ncg = len(cgs)
        w_b = np.stack([_ceil4(cg["nb"]) for cg in cgs])        # (ncg, 32)
        core_w = w_b[:, assign.reshape(-1)].reshape(ncg, 8, 4).sum(2)
        self.wcg = core_w.max(1)                                 # (ncg,)

        # ---- pack cg columns bin-major into banks of 512 ----
        pieces = []          # (cg, bin, off, lo, hi) covering cg-local [lo,hi)
        bin_i, off = 0, 0
        for ci in range(ncg):
            w = int(self.wcg[ci])
            lo = 0
            while w > 0:
                take = min(w, BANK - off)
                pieces.append((ci, bin_i, off, lo, lo + take))
                off += take
                lo += take
                w -= take
                if off == BANK:
                    bin_i, off = bin_i + 1, 0
        if off > 0:
            ci, bi, o, lo, hi = pieces[-1]
            pieces[-1] = (ci, bi, o, lo, hi + (BANK - off))   # pad-extend
            bin_i += 1
        self.nbins = bin_i
        self.rtot = self.nbins * BANK // 2    # pair partials

        # group sizes: ramp [1, 2, 4, 4, ...] for fast pipeline fill
        sizes = []
        left = self.nbins
        for want in (1, 2) if self.nbins > 8 else ():
            sizes.append(min(want, left))
            left -= sizes[-1]
        while left > 0:
            sizes.append(min(4, left))
            left -= sizes[-1]
        self.ngroups = len(sizes)
        self.nbg = sizes
        cum = np.concatenate([[0], np.cumsum(sizes)])
        bin_g = np.searchsorted(cum, np.arange(self.nbins), side="right") - 1
        bin_beta = np.arange(self.nbins) - cum[bin_g]

        # ---- band-major DRAM layouts ----
        # wmv: bin -> dram col block, ordered by (beta, g)
        order = sorted(range(self.nbins), key=lambda bi: (bin_beta[bi],
                                                          bin_g[bi]))
        self.wmv_dbase = np.empty(self.nbins, np.int64)
        for k, bi in enumerate(order):
            self.wmv_dbase[bi] = k * BANK
        self.wtot = self.nbins * BANK

        # stationary slots: sbuf per-band sequential; dram band-major
        self.band_nslot = [0, 0, 0, 0]
        piece_slot = []
        piece_gn = [{}, {}, {}, {}]      # beta -> {g: nslots}
        for ci, bi, o, lo, hi in pieces:
            beta = int(bin_beta[bi])
            g = int(bin_g[bi])
            piece_slot.append(self.band_nslot[beta])
            self.band_nslot[beta] += 1
            piece_gn[beta][g] = piece_gn[beta].get(g, 0) + 1
        xbase = np.concatenate([[0], np.cumsum(self.band_nslot)])
        self.xst_fill = []   # (dram_slot, ci)
        self.mm = [[] for _ in range(self.ngroups)]  # (beta, off, w, bslot)
        for k, (ci, bi, o, lo, hi) in enumerate(pieces):
            g, beta = int(bin_g[bi]), int(bin_beta[bi])
            bslot = piece_slot[k]
            dslot = int(xbase[beta]) + bslot
            self.xst_fill.append((dslot, ci))
            self.mm[g].append((beta, o, hi - lo, bslot))
        self.nslots = int(xbase[4])
        self.xtot = self.nslots * 128
        for g in range(self.ngroups):
            self.mm[g].sort(key=lambda t: (t[1], t[0]))

        # DMA pieces per band: group ranges [0],[1],[2,3],[4..]
        # xst entries (beta, dslot0, nslots, bslot0); wmv entries
        # (beta, dcol0, ncols, g0, ngrange)
        self.xst_dma = [[] for _ in range(4)]
        self.wmv_dma = [[] for _ in range(4)]
        for beta in range(4):
            gs = sorted(piece_gn[beta])          # groups present
            gcuts = [g for g in (gs[0], gs[0] + 1, gs[0] + 2, gs[0] + 4)
                     if g <= gs[-1]] + [gs[-1] + 1]
            gcuts = sorted(set(gcuts))
            # xst pieces
            spos = 0
            for a, b in zip(gcuts, gcuts[1:]):
                n = sum(piece_gn[beta].get(g, 0) for g in range(a, b))
                if n > 0:
                    self.xst_dma[beta].append(
                        (beta, int(xbase[beta]) + spos, n, spos))
                    spos += n
            # wmv pieces (band has one bank per present group, g-contig)
            dbase = None
            for bi in range(self.nbins):
                if int(bin_beta[bi]) == beta:
                    dbase = int(self.wmv_dbase[bi])
                    break
            for a, b in zip(gcuts, gcuts[1:]):
                ng = sum(1 for g in range(a, b) if g in piece_gn[beta])
                if ng > 0:
                    off_g = a - gs[0]
                    self.wmv_dma[beta].append(
                        (beta, dbase + off_g * BANK, ng * BANK, a, ng))

        # cg-local col -> (wmv dram col, psum col)
        self.gcol = [np.empty(int(self.wcg[ci]), np.int64)
                     for ci in range(ncg)]
        self.pcol = [np.empty(int(self.wcg[ci]), np.int64)
                     for ci in range(ncg)]
        for ci, bi, o, lo, hi in pieces:
            n = min(hi, int(self.wcg[ci])) - lo
            if n > 0:
                ar = np.arange(n)
                self.gcol[ci][lo:lo + n] = self.wmv_dbase[bi] + o + ar
                self.pcol[ci][lo:lo + n] = bi * BANK + o + ar

        # R cumulative base per group (pair cols)
        self.rbase = [int(cum[g]) * BANK // 2 for g in range(self.ngroups + 1)]

        # ---- per-core unpack runs (reduceat over pair cols) ----
        self.runs = []
        for core in range(8):
            bs = assign[core]
            ev = []
            for ci, cg in enumerate(cgs):
                pos = 0
                for lb in range(4):
                    n = int(cg["nb"][bs[lb]])
                    if n == 0:
                        continue
                    k = _ceil4(n) // GRAN
                    first = True
                    p0 = pos // GRAN
                    while k > 0:
                        gc = self.pcol[ci][p0 * GRAN]
                        run = 1
                        while (run < k and
                               self.pcol[ci][(p0 + run) * GRAN] ==
                               gc + run * GRAN):
                            run += 1
                        ev.append((int(gc) // 2, run * GRAN // 2,
                                   ci * 4 + lb, first))
                        first = False
                        p0 += run
                        k -= run
                    pos += _ceil4(n)
            ev.sort()
            starts, owners, firsts = [], [], []
            cur_end = 0
            for st, ln, ow, fi in ev:
                if st > cur_end:
                    starts.append(cur_end)
                    owners.append(-1)
                    firsts.append(True)
                starts.append(st)
                owners.append(ow)
                firsts.append(fi)
                cur_end = st + ln
            if cur_end < self.rtot:
                starts.append(cur_end)
                owners.append(-1)
                firsts.append(True)
            self.runs.append((np.array(starts), np.array(owners),
                              np.array(firsts)))

        h = hashlib.md5(repr((NK, THR_LN, self.nbins, tuple(sizes),
                              tuple(self.wcg.tolist()),
                              tuple(pieces))).encode()).hexdigest()
        self.key = h


def _build(minus_c, sched):
    import concourse.tile as tile
    from concourse import bacc, mybir

    fp32 = mybir.dt.float32
    fp16 = mybir.dt.float16
    bf16 = mybir.dt.bfloat16
    AF = mybir.ActivationFunctionType
    OP = mybir.AluOpType
    import concourse.bass as bass

    nc = bacc.Bacc("TRN2", target_bir_lowering=False, debug=False,
                   num_devices=NCORES)

    d_xst = nc.dram_tensor("xst", [NK, sched.xtot], bf16,
                           kind="ExternalInput")
    d_wmv = nc.dram_tensor("wmv", [NK, sched.wtot], bf16,
                           kind="ExternalInput")
    d_out = nc.dram_tensor("out", [128, sched.rtot], fp16,
                           kind="ExternalOutput")

    ngroups = sched.ngroups
    PMAX = 96 + NK

    with tile.TileContext(nc) as tc:
        with (
            tc.tile_pool(name="const", bufs=1) as cp,
            tc.tile_pool(name="ering", bufs=3) as ep,
            tc.tile_pool(name="psS", bufs=2, space=bass.MemorySpace.PSUM) as psS,
        ):
            warm = cp.tile([1, 2], fp32, tag="warm")
            nc.vector.memset(warm[:, :], 0.0)
            zb = cp.tile([128, 1], fp32, tag="zb")
            nc.vector.memset(zb[:, :], 0.0)
            nc.scalar.activation(warm[:, 0:1], warm[:, 1:2], AF.Exp,
                                 bias=zb[0:1, 0:1])

            xst = cp.tile([PMAX, max(sched.band_nslot) * 128], bf16,
                          tag="xst")
            wmv = cp.tile([PMAX, ngroups * BANK], bf16, tag="wmv")
            R = cp.tile([128, sched.rtot], fp16, tag="R")

            def x_issue(eng, x):
                beta, d0, n, b0 = x
                eng.dma_start(
                    xst[32 * beta:32 * beta + NK,
                        b0 * 128:(b0 + n) * 128],
                    d_xst.ap()[:, d0 * 128:(d0 + n) * 128])

            def w_issue(eng, wx):
                beta, d0, nc_, g0, ng = wx
                eng.dma_start(
                    wmv[32 * beta:32 * beta + NK,
                        g0 * BANK:(g0 + ng) * BANK],
                    d_wmv.ap()[:, d0:d0 + nc_])

            # piece-index-major issue: all [g0] pieces, then [g1], ...
            # early pieces spread over sync/gpsimd/scalar, rest over 2
            q3 = [nc.sync, nc.gpsimd, nc.scalar]
            q2 = [nc.sync, nc.gpsimd]
            k = 0
            for pi in range(4):
                for beta in range(4):
                    for lst in (sched.xst_dma, sched.wmv_dma):
                        if pi < len(lst[beta]):
                            if pi < 2:
                                eng = q3[k % 3]
                            else:
                                eng = q2[k % 2]
                            k += 1
                            if lst is sched.xst_dma:
                                x_issue(eng, lst[beta][pi])
                            else:
                                w_issue(eng, lst[beta][pi])

            ocur = 0
            oi = 0
            for g in range(ngroups):
                nbg = sched.nbg[g]
                h = nbg * BANK
                sS = psS.tile([128, 4 * BANK], fp32, tag="S", name="sS")
                for beta, off, w, bslot in sched.mm[g]:
                    r0 = 32 * beta
                    nc.tensor.matmul(
                        sS[:, beta * BANK + off:beta * BANK + off + w],
                        xst[r0:r0 + NK, bslot * 128:(bslot + 1) * 128],
                        wmv[r0:r0 + NK, g * BANK + off:g * BANK + off + w],
                        start=True, stop=True,
                        tile_position=(r0, 0),
                    )
                E = ep.tile([128, 4 * BANK], fp16, tag="E", name="E")
                nc.scalar.activation(E[:, 0:h], sS[:, 0:h], AF.Exp,
                                     scale=float(minus_c), bias=zb[:, 0:1])
                Ev = E[:, 0:h].rearrange("p (s x) -> p s x", x=GRAN)
                rb = sched.rbase[g]
                Rp = (R[:, rb:rb + h // 2]
                      .rearrange("p (s x) -> p s x", x=GRAN // 2))
                half = GRAN // 2
                nc.vector.tensor_tensor(Rp, Ev[:, :, 0:half],
                                        Ev[:, :, half:GRAN], OP.add)
                rend = sched.rbase[g + 1]
                if g % 2 == 1 or g == ngroups - 1:
                    # outputs on sync only: keeping gpsimd's software-DGE
                    # rings idle after the input stream shortens its
                    # expensive end-of-program dge_drain
                    nc.sync.dma_start(d_out.ap()[:, ocur:rend],
                                      R[:, ocur:rend])
                    oi += 1
                    ocur = rend

    nc.compile()
    return nc


def _get_nc(minus_c, sched):
    key = (float(minus_c), sched.key)
    if key not in _cache:
        _cache[key] = _build(minus_c, sched)
    return _cache[key]


def _make_schedule(c, q, xs):
    """Pieces -> chunk-groups -> batch assignment -> Schedule."""
    thr = np.sqrt(THR_LN / c)
    pieces = []
    for f in range(F):
        for t in range(V // T):
            xw = xs[t * T:(t + 1) * T, f]
            pieces.append((float(xw.min() + xw.max()) / 2, f, t))
    pieces.sort()
    cgs = []
    for i in range(0, len(pieces), S):
        grp = pieces[i:i + S]
        los, his = [], []
        for _, f, t in grp:
            xw = xs[t * T:(t + 1) * T, f]
            los.append(xw.min())
            his.append(xw.max())
        lo, hi = min(los) - thr, max(his) + thr
        sel = (q >= lo) & (q <= hi)
        nb = sel.sum(axis=1).astype(np.int64)
        if nb.max() == 0:
            continue
        cgs.append({"m": (lo + hi) / 2, "fs": [f for _, f, _ in grp],
                    "ts": [t for _, _, t in grp], "sel": sel, "nb": nb})

    # batch -> core assignment: minimize sum_cg max_core width
    w_b = np.stack([_ceil4(cg["nb"]) for cg in cgs])            # (ncg, 32)
    tot = w_b.sum(0)
    order = np.argsort(-tot, kind="stable")
    assign = [[] for _ in range(8)]
    loads = np.zeros(8)
    for b in order:
        ci = int(np.argmin([loads[i] if len(assign[i]) < 4 else 1e18
                            for i in range(8)]))
        assign[ci].append(int(b))
        loads[ci] += tot[b]
    assign = np.array(assign)

    def obj(a):
        return w_b[:, a.reshape(-1)].reshape(-1, 8, 4).sum(2).max(1).sum()

    rng = np.random.RandomState(0)
    best = obj(assign)
    for _ in range(20000):
        c1, c2 = rng.randint(8), rng.randint(8)
        if c1 == c2:
            continue
        i1, i2 = rng.randint(4), rng.randint(4)
        a2 = assign.copy()
        a2[c1, i1], a2[c2, i2] = assign[c2, i2], assign[c1, i1]
        o2 = obj(a2)
        if o2 < best:
            best, assign = o2, a2
    return Schedule(cgs, assign)


def kernel(q2_obs_scaled, amplitude_logits, volumes, filters, sigma,
           _trace=False, _tmpdir=None):
    from concourse.bass_utils import run_bass_kernel_spmd

    sig = float(np.asarray(sigma).reshape(()))
    minus_c = -0.5 / (sig + 0.001) ** 2
    c = -minus_c

    q = np.asarray(q2_obs_scaled, np.float64)                    # (B, P)
    lg = np.asarray(amplitude_logits, np.float64).reshape(B, F, P)
    vol = np.asarray(volumes, np.float64).reshape(V)
    fil = np.asarray(filters, np.float64).reshape(F)

    mx = lg.max(axis=2, keepdims=True)
    lnamp = lg - (mx + np.log(np.exp(lg - mx).sum(axis=2, keepdims=True)))

    vperm = np.argsort(vol, kind="stable")
    vs = vol[vperm]
    xs = vs[:, None] * fil[None, :]                              # (V, F)

    sched = _make_schedule(c, q, xs)
    nc = _get_nc(minus_c, sched)
    cgs = sched.cgs
    ncg = len(cgs)

    # ---- stationary tile (shared by all cores) ----
    xst = np.zeros((NK, sched.xtot), dtype=BF16)
    ones_j = np.zeros((S, 128), dtype=BF16)
    for j in range(S):
        ones_j[j, j * T:(j + 1) * T] = 1.0
    xrows_cg = {}
    for ci, cg in enumerate(cgs):
        X = np.concatenate([xs[t * T:(t + 1) * T, f] - cg["m"]
                            for f, t in zip(cg["fs"], cg["ts"])])
        X2h, X2m = _split2(X * X)
        Xh, Xm = _split2(X)
        xrows_cg[ci] = (X2h, X2m, Xh, Xm)
    for dslot, ci in sched.xst_fill:
        X2h, X2m, Xh, Xm = xrows_cg[ci]
        c0 = dslot * 128
        xst[0, c0:c0 + 128] = X2h
        xst[1, c0:c0 + 128] = X2m
        xst[2, c0:c0 + 128] = Xh
        xst[3, c0:c0 + 128] = Xh
        xst[4, c0:c0 + 128] = Xm
        xst[5, c0:c0 + 128] = 1.0
        xst[6, c0:c0 + 128] = 1.0
        for j in range(S):
            xst[7 + j, c0:c0 + 128] = ones_j[j]

    # ---- per-cg moving data for all 32 batches ----
    cg_data = []
    for ci, cg in enumerate(cgs):
        bi_, pi_ = np.nonzero(cg["sel"])          # b-major, p ascending
        Q = q[bi_, pi_] - cg["m"]
        Wh, Wm = _split2(-2.0 * Q)
        Q2h, Q2m = _split2(Q * Q)
        Ls = [(-lnamp[bi_, f, pi_] / c).astype(BF16).astype(np.float32)
              for f in cg["fs"]]
        off_b = np.zeros(B + 1, np.int64)
        np.cumsum(np.bincount(bi_, minlength=B), out=off_b[1:])
        cg_data.append((Wh, Wm, Q2h, Q2m, Ls, off_b))

    # ---- per-core moving tiles ----
    in_maps = []
    for core in range(NCORES):
        wmv = np.zeros((NK, sched.wtot), dtype=BF16)
        wmv[0:2] = 1.0
        wmv[5:NK] = PAD_PHI
        bs = sched.assign[core]
        for ci in range(ncg):
            Wh, Wm, Q2h, Q2m, Ls, off_b = cg_data[ci]
            gcol = sched.gcol[ci]
            pos = 0
            for lb in range(4):
                b = int(bs[lb])
                n = int(cgs[ci]["nb"][b])
                if n == 0:
                    continue
                seg = slice(off_b[b], off_b[b] + n)
                gc = gcol[pos:pos + n]
                wmv[2, gc] = Wh[seg]
                wmv[3, gc] = Wm[seg]
                wmv[4, gc] = Wh[seg]
                wmv[5, gc] = Q2h[seg]
                wmv[6, gc] = Q2m[seg]
                for j in range(S):
                    wmv[7 + j, gc] = Ls[j][seg]
                pos += _ceil4(n)
        in_maps.append({"xst": xst, "wmv": wmv})

    kw = {}
    if _trace:
        kw = {"trace": True, "tmpdir": _tmpdir}
    res = run_bass_kernel_spmd(nc, in_maps, core_ids=list(range(NCORES)), **kw)

    # ---- host unpack ----
    dest = np.empty((ncg, 128), np.int64)
    for ci, cg in enumerate(cgs):
        for j, (f, t) in enumerate(zip(cg["fs"], cg["ts"])):
            dest[ci, j * T:(j + 1) * T] = vperm[t * T:(t + 1) * T] * F + f
    out = np.zeros((B, V * F), dtype=np.float64)
    for core in range(NCORES):
        Rr = np.asarray(res.results[core]["out"], np.float16)
        P32 = Rr.astype(np.float32)
        starts, owners, firsts = sched.runs[core]
        red = np.add.reduceat(P32, starts, axis=1)
        vals = np.zeros((128, ncg * 4), np.float32)
        dm = (owners >= 0) & firsts
        vals[:, owners[dm]] = red[:, dm]
        for r in np.nonzero((owners >= 0) & ~firsts)[0]:
            vals[:, owners[r]] += red[:, r]
        v3 = vals.reshape(128, ncg, 4)
        bs = sched.assign[core]
        for lb in range(4):
            out[bs[lb], dest.reshape(-1)] = v3[:, :, lb].T.reshape(-1)
    out = out.reshape(B, V, F).astype(np.float32)
    if _trace:
        return out, res
    return out
